# revision 13
# baseline (speedup 1.0000x reference)
import math
import sys

sys.path.insert(0, "/opt/trn_rl_repo")

import numpy as np
import ml_dtypes

bf16np = ml_dtypes.bfloat16

# ---------------- problem constants (hardcoded; kernel.py must be self-contained) ----
B, T, S, D, H, L, DFF, IN, PERIOD = 16, 600, 600, 1024, 8, 8, 4096, 52, 25
HD = D // H          # 128
NC = 8               # cores
BC = B // NC         # 2 batches per core
DI = D // 128        # 8 i-tiles
DFI = DFF // 128     # 32
INV = 1.0 / math.sqrt(HD)
TK = 1 + T           # 601 keys (adapter + T)
# chunks of the token dim (>=256 wide keeps LDWEIGHTS hidden; <=512 fits a psum bank)
TCH = [(0, 300), (300, 300)]
KCH = [(0, 301), (301, 300)]         # 601-wide
KT = [(0, 128), (128, 128), (256, 128), (384, 128), (512, 89)]  # key tiles of 601
SLOPES = [0.5 ** (h + 1) for h in range(H)]

_cache = {}


def _build():
    """Build the per-core Bass graph (SPMD; same program all 8 cores)."""
    from concourse import bacc, mybir
    import concourse.bass as bass
    import concourse.tile as tile

    f32 = mybir.dt.float32
    bf = mybir.dt.bfloat16
    i32 = mybir.dt.int32
    AF = mybir.ActivationFunctionType
    OP = mybir.AluOpType

    nc = bacc.Bacc("TRN2", target_bir_lowering=False, debug=False, num_devices=NC)

    def din(name, shape, dt=f32):
        return nc.dram_tensor(name, shape, dt, kind="ExternalInput").ap()

    # ---- DRAM inputs (host-prepped layouts; *_c = tile-contiguous) ----
    xT = din("xT", [BC, IN, T], bf)              # x transposed, bf16
    memT_c = din("memT_c", [BC, DI, 128, T], bf)
    tsf = din("tsf", [1, BC])                    # timesteps as f32
    efm = din("efm", [128, DI])                  # e/(2pi) tiled per i-tile col
    phs = din("phs", [128, DI])                  # phase (0 / .25)
    peT_c = din("peT_c", [DI, 128, T])           # pe.T + b_in  (f32)
    w_inT = din("w_inT", [IN, D], bf)
    te_w1T_c = din("te_w1T_c", [DI, 128, D], bf)
    te_w2T_c = din("te_w2T_c", [DI, 128, D], bf)
    te_b1t = din("te_b1t", [128, DI])
    te_b2t = din("te_b2t", [128, DI])
    sa_wqkvT_c = din("sa_wqkvT_c", [L, 3, DI, 128, D], bf)
    sa_bqkvt = din("sa_bqkvt", [L, 128, 3 * DI])  # pre-tiled [128, 24]; q-part prescaled by INV
    sa_bvrow = din("sa_bvrow", [L, 1, D], bf)     # v-bias as row (for ones-MM trick)
    sa_woT_c = din("sa_woT_c", [L, DI, 128, D], bf)
    sa_bot = din("sa_bot", [L, 128, DI])
    ca_wqkvT_c = din("ca_wqkvT_c", [L, 3, DI, 128, D], bf)
    ca_bqkvt = din("ca_bqkvt", [L, 128, 3 * DI])
    ca_woT_c = din("ca_woT_c", [L, DI, 128, D], bf)
    ca_bot = din("ca_bot", [L, 128, DI])
    ff_w1T_c = din("ff_w1T_c", [L, 4, DI, 128, D], bf)
    ff_b1t = din("ff_b1t", [L, 128, DFI])
    ff_w2T_c = din("ff_w2T_c", [L, 2, 16, 128, D], bf)
    ff_b2t = din("ff_b2t", [L, 128, DI])
    lngt = din("lngt", [L, 3, 128, DI])
    lnbt = din("lnbt", [L, 3, 128, DI])
    steps_d = din("steps_d", [5, 128, T], bf)    # alibi steps, kt-tiled [k, q]
    w_outT_c = din("w_outT_c", [DI, 128, IN], bf)
    b_out = din("b_out", [IN, 1])
    out_d = nc.dram_tensor("out", [BC, IN, T], f32, kind="ExternalOutput").ap()

    with tile.TileContext(nc) as tc:
        res = tc.alloc_tile_pool(name="res", bufs=1)      # persistent
        w6 = tc.alloc_tile_pool(name="w6", bufs=26)       # bf16 [128,601] q/k/pt/attn ws
        fa = tc.alloc_tile_pool(name="fa", bufs=17)       # bf16 [128,601] ffa tiles
        hqp = tc.alloc_tile_pool(name="hqp", bufs=10)     # bf16 [128,601] LN targets
        vs = tc.alloc_tile_pool(name="vs", bufs=6)        # bf16 [128,1024] V tiles
        f6 = tc.alloc_tile_pool(name="f6", bufs=7)        # f32 [128,600] workspace
        sqp = tc.alloc_tile_pool(name="sqp", bufs=8)      # bf16 [128,600] LN squares
        wg = tc.alloc_tile_pool(name="wg", bufs=21)       # bf16 [128,1024] weights
        sm = tc.alloc_tile_pool(name="sm", bufs=1)        # small persistents
        pk = tc.alloc_tile_pool(name="pk", bufs=8, space="PSUM")

        # persistent bf16 residual stream (adapter col 0) - ONE batch at a time
        hb1 = [res.tile([128, TK], bf, tag=f"hb_{i}", name=f"hb_{i}") for i in range(DI)]
        hb = [hb1 for _ in range(BC)]
        mx1 = [res.tile([128, TK], bf, tag=f"mx_{i}", name=f"mx_{i}") for i in range(DI)]

        ones_b = sm.tile([1, 128], bf, tag="ones_b", name="ones_b")
        nc.vector.memset(ones_b[:], 1.0)
        ones_b128 = sm.tile([128, 128], bf, tag="ones_b128", name="ones_b128")
        nc.vector.memset(ones_b128[:], 1.0)
        ones_f = sm.tile([1, 128], f32, tag="ones_f", name="ones_f")
        nc.vector.memset(ones_f[:], 1.0)
        eft = sm.tile([128, DI], f32, tag="eft", name="eft")
        nc.sync.dma_start(eft[:], efm[:])
        pht = sm.tile([128, DI], f32, tag="pht", name="pht")
        nc.sync.dma_start(pht[:], phs[:])
        tst = sm.tile([1, BC], f32, tag="tst", name="tst")
        eps_t = sm.tile([128, 1], f32, tag="eps_t", name="eps_t")
        nc.vector.memset(eps_t[:], 1e-5)
        nc.sync.dma_start(tst[:], tsf[:])
        adp = [sm.tile([128, BC], bf, tag=f"adp{i}", name=f"adp{i}") for i in range(DI)]  # adapter bf16
        steps_t = [sm.tile([128, T], bf, tag=f"steps{k}", name=f"steps{k}") for k in range(5)]
        for k in range(5):
            nc.sync.dma_start(steps_t[k][:], steps_d[k])

        # ---------- timestep embedding ----------
        ptb = pk.tile([128, BC], f32, tag="pk", name="ptb")
        nc.tensor.matmul(ptb[:], ones_f[:], tst[:], start=True, stop=True)  # t bcast f32
        temb = []
        for i in range(DI):
            y = sm.tile([128, BC], f32, tag=f"y{i}", name=f"y{i}")
            nc.vector.tensor_scalar_mul(y[:], ptb[:], eft[:, i : i + 1])
            nc.vector.tensor_scalar_add(y[:], y[:], pht[:, i : i + 1])
            yi = sm.tile([128, BC], i32, tag=f"yi{i}", name=f"yi{i}")
            nc.vector.tensor_copy(yi[:], y[:])
            yr = sm.tile([128, BC], f32, tag=f"yr{i}", name=f"yr{i}")
            nc.vector.tensor_copy(yr[:], yi[:])
            fr = sm.tile([128, BC], f32, tag=f"fr{i}", name=f"fr{i}")
            nc.vector.tensor_sub(fr[:], y[:], yr[:])
            tb = sm.tile([128, BC], bf, tag=f"tb{i}", name=f"tb{i}")
            nc.scalar.activation(tb[:], fr[:], AF.Sin, scale=2 * math.pi)
            temb.append(tb)

        def mlp1024(wT_c, bt_d, ins, act, outs_tag):
            """[D,D] proj on BC-wide f-major input tiles. Returns 8 bf16 [128,BC] tiles."""
            bt = sm.tile([128, DI], f32, tag=outs_tag + "_b", name=outs_tag + "_b")
            nc.sync.dma_start(bt[:], bt_d[:])
            ws = []
            for i in range(DI):
                w = wg.tile([128, D], bf, tag="wg", name="wg")
                nc.sync.dma_start(w[:], wT_c[i])
                ws.append(w)
            outs = []
            for o in range(DI):
                p = pk.tile([128, BC], f32, tag="pk", name="pmlp")
                for i in range(DI):
                    nc.tensor.matmul(p[:], ws[i][:, o * 128 : (o + 1) * 128], ins[i][:],
                                     start=(i == 0), stop=(i == DI - 1))
                ob = sm.tile([128, BC], bf, tag=f"{outs_tag}{o}", name=f"{outs_tag}{o}")
                nc.scalar.activation(ob[:], p[:], act, bias=bt[:, o : o + 1])
                outs.append(ob)
            return outs

        z1 = mlp1024(te_w1T_c, te_b1t, temb, AF.Silu, "z1")
        z2 = mlp1024(te_w2T_c, te_b2t, z1, AF.Identity, "z2")
        for i in range(DI):
            nc.vector.tensor_copy(adp[i][:], z2[i][:])

        # ---------- helpers ----------
        def layernorm(b, g_ap, b_ap, tgt=None):
            """LN over features of X held in hb[b][:, 1:] (bf16). Sums via ones-matmul
            reduction. If tgt given, apply writes tgt and hb is back-filled off the
            critical path; else apply is in place on hb."""
            sq = []
            for o in range(DI):
                s = sqp.tile([128, T], bf, tag="sqp", name="sqp")
                nc.vector.tensor_mul(s[:], hb[b][o][:, 1:], hb[b][o][:, 1:])
                sq.append(s)
            m = f6.tile([128, T], f32, tag="f6", name="f6")
            rstd = f6.tile([128, T], f32, tag="f6", name="f6")
            mr = f6.tile([128, T], f32, tag="f6", name="f6")
            for c0, cw in TCH:
                pS = pk.tile([128, 512], f32, tag="pk", name="pk")
                for o in range(DI):
                    nc.tensor.matmul(pS[:, :cw], ones_b128[:],
                                     hb[b][o][:, 1 + c0 : 1 + c0 + cw],
                                     start=(o == 0), stop=(o == DI - 1))
                pS2 = pk.tile([128, 512], f32, tag="pk", name="pk")
                for o in range(DI):
                    nc.tensor.matmul(pS2[:, :cw], ones_b128[:], sq[o][:, c0 : c0 + cw],
                                     start=(o == 0), stop=(o == DI - 1))
                nc.vector.tensor_scalar_mul(m[:, c0 : c0 + cw], pS[:, :cw], 1.0 / D)
                m2 = f6.tile([128, T], f32, tag="f6", name="f6")
                nc.vector.tensor_tensor(m2[:, :cw], m[:, c0 : c0 + cw], m[:, c0 : c0 + cw], OP.mult)
                var = f6.tile([128, T], f32, tag="f6", name="f6")
                nc.vector.scalar_tensor_tensor(var[:, :cw], pS2[:, :cw], 1.0 / D, m2[:, :cw],
                                               OP.mult, OP.subtract)
                sd = f6.tile([128, T], f32, tag="f6", name="f6")
                nc.scalar.activation(sd[:, :cw], var[:, :cw], AF.Sqrt, bias=eps_t[:])
                nc.vector.reciprocal(rstd[:, c0 : c0 + cw], sd[:, :cw])
                nc.vector.tensor_tensor(mr[:, c0 : c0 + cw], m[:, c0 : c0 + cw],
                                        rstd[:, c0 : c0 + cw], OP.mult)
            for o in range(DI):
                dst = tgt[o] if tgt is not None else None
                for c0, cw in TCH:
                    t1 = f6.tile([128, T], f32, tag="f6", name="f6")
                    nc.vector.tensor_tensor(t1[:, :cw], hb[b][o][:, 1 + c0 : 1 + c0 + cw],
                                            rstd[:, c0 : c0 + cw], OP.mult)
                    nc.vector.tensor_tensor(t1[:, :cw], t1[:, :cw], mr[:, c0 : c0 + cw],
                                            OP.subtract)
                    if dst is not None:
                        nc.scalar.activation(dst[:, c0 : c0 + cw], t1[:, :cw], AF.Identity,
                                             bias=b_ap[:, o : o + 1], scale=g_ap[:, o : o + 1])
                    else:
                        nc.scalar.activation(hb[b][o][:, 1 + c0 : 1 + c0 + cw], t1[:, :cw],
                                             AF.Identity,
                                             bias=b_ap[:, o : o + 1], scale=g_ap[:, o : o + 1])
                if dst is not None:
                    nc.vector.tensor_copy(hb[b][o][:, 1:], dst[:, :T])

        def proj_res(b, wT_c_l, bot_ap, rhs_tiles):
            """out-proj [D,D] + bias + residual into hf[b] (X pre-LN)."""
            ws = []
            for i in range(DI):
                w = wg.tile([128, D], bf, tag="wg", name="wg")
                nc.sync.dma_start(w[:], wT_c_l[i])
                ws.append(w)
            for o in range(DI):
                for c0, cw in TCH:
                    p = pk.tile([128, 512], f32, tag="pk", name="pk")
                    for i in range(DI):
                        nc.tensor.matmul(p[:, :cw], ws[i][:, o * 128 : (o + 1) * 128],
                                         rhs_tiles[i][:, c0 : c0 + cw],
                                         start=(i == 0), stop=(i == DI - 1))
                    nc.vector.scalar_tensor_tensor(hb[b][o][:, 1 + c0 : 1 + c0 + cw], p[:, :cw],
                                                   bot_ap[:, o : o + 1],
                                                   hb[b][o][:, 1 + c0 : 1 + c0 + cw],
                                                   OP.add, OP.add)

        # per-layer bias tiles (re-DMAed each (b, l))
        sa_bq = sm.tile([128, 3 * DI], f32, tag="sa_bq", name="sa_bq")
        ca_bq = sm.tile([128, 3 * DI], f32, tag="ca_bq", name="ca_bq")
        sa_bo_t = sm.tile([128, DI], f32, tag="sa_bo_t", name="sa_bo_t")
        ca_bo_t = sm.tile([128, DI], f32, tag="ca_bo_t", name="ca_bo_t")
        f_b1 = sm.tile([128, DFI], f32, tag="f_b1", name="f_b1")
        f_b2 = sm.tile([128, DI], f32, tag="f_b2", name="f_b2")
        lng = [sm.tile([128, DI], f32, tag=f"lng{k}", name=f"lng{k}") for k in range(3)]
        lnb = [sm.tile([128, DI], f32, tag=f"lnb{k}", name=f"lnb{k}") for k in range(3)]
        bvr = sm.tile([1, D], bf, tag="bvr", name="bvr")
        bo_t = sm.tile([IN, 1], f32, tag="bo_t", name="bo_t")
        nc.sync.dma_start(bo_t[:], b_out[:])

        # ================= batch-serial main =================
        for b in range(BC):
            # ---------- input projection + pe ----------
            xb = sm.tile([IN, T], bf, tag="xb", name="xb")
            nc.sync.dma_start(xb[:], xT[b])
            w_in_t = sm.tile([IN, D], bf, tag="w_in_t", name="w_in_t")
            nc.sync.dma_start(w_in_t[:], w_inT[:])
            for o in range(DI):
                pe_t = f6.tile([128, T], f32, tag="f6", name="f6")
                nc.sync.dma_start(pe_t[:], peT_c[o])
                for c0, cw in TCH:
                    p = pk.tile([128, 512], f32, tag="pk", name="pk")
                    nc.tensor.matmul(p[:, :cw], w_in_t[:, o * 128 : (o + 1) * 128],
                                     xb[:, c0 : c0 + cw], start=True, stop=True)
                    nc.vector.tensor_tensor(hb[b][o][:, 1 + c0 : 1 + c0 + cw], p[:, :cw],
                                            pe_t[:, c0 : c0 + cw], OP.add)
                nc.vector.tensor_copy(hb[b][o][:, 0:1], adp[o][:, b : b + 1])
                nc.sync.dma_start(mx1[o][:, 1:], memT_c[b, o])
                nc.vector.tensor_copy(mx1[o][:, 0:1], adp[o][:, b : b + 1])

            for l in range(L):
                nc.sync.dma_start(sa_bq[:], sa_bqkvt[l])
                nc.sync.dma_start(ca_bq[:], ca_bqkvt[l])
                nc.sync.dma_start(sa_bo_t[:], sa_bot[l])
                nc.sync.dma_start(ca_bo_t[:], ca_bot[l])
                nc.sync.dma_start(f_b1[:], ff_b1t[l])
                nc.sync.dma_start(f_b2[:], ff_b2t[l])
                for k in range(3):
                    nc.sync.dma_start(lng[k][:], lngt[l, k])
                    nc.sync.dma_start(lnb[k][:], lnbt[l, k])
                nc.sync.dma_start(bvr[:], sa_bvrow[l])

                # ================= self-attention =================
                def sa_proj(mat, src_off, chunks, bias_off, scale):
                    ws = []
                    for i in range(DI):
                        w = wg.tile([128, D], bf, tag="wg", name="wg")
                        nc.sync.dma_start(w[:], sa_wqkvT_c[l, mat, i])
                        ws.append(w)
                    outs = []
                    for ot in range(DI):
                        dst = w6.tile([128, TK], bf, tag="w6", name="w6")
                        for c0, cw in chunks:
                            p = pk.tile([128, 512], f32, tag="pk", name="pk")
                            for i in range(DI):
                                nc.tensor.matmul(p[:, :cw], ws[i][:, ot * 128 : (ot + 1) * 128],
                                                 hb[b][i][:, src_off + c0 : src_off + c0 + cw],
                                                 start=(i == 0), stop=(i == DI - 1))
                            nc.scalar.activation(dst[:, c0 : c0 + cw], p[:, :cw], AF.Identity,
                                                 bias=sa_bq[:, bias_off + ot : bias_off + ot + 1],
                                                 scale=scale)
                        outs.append(dst)
                    return outs

                qsb = sa_proj(0, 1, TCH, 0, INV)
                ksb = sa_proj(1, 0, KCH, DI, 1.0)
                # V transposed ([key, head*hd]) via hx-stationary matmuls
                vw = []
                for i in range(DI):
                    w = wg.tile([128, D], bf, tag="wg", name="wg")
                    nc.sync.dma_start(w[:], sa_wqkvT_c[l, 2, i])
                    vw.append(w)
                vsb = [vs.tile([128, 1024], bf, tag="vs", name="vs") for _ in range(5)]
                for vc in range(2):
                    vc0 = vc * 512
                    for kt_i, (k0, kwd) in enumerate(KT):
                        p = pk.tile([128, 512], f32, tag="pk", name="pk")
                        for i in range(DI):
                            nc.tensor.matmul(p[:kwd, :], hb[b][i][:, k0 : k0 + kwd],
                                             vw[i][:, vc0 : vc0 + 512],
                                             start=(i == 0), stop=False)
                        nc.tensor.matmul(p[:kwd, :], ones_b[:, :kwd],
                                         bvr[:, vc0 : vc0 + 512], start=False, stop=True)
                        nc.scalar.copy(vsb[kt_i][:kwd, vc0 : vc0 + 512], p[:kwd, :])

                attn = []
                for h in range(H):
                    pts = []
                    for kt_i, (k0, kwd) in enumerate(KT):
                        sx = f6.tile([128, T], f32, tag="f6", name="f6")
                        for c0, cw in TCH:
                            p = pk.tile([128, 512], f32, tag="pk", name="pk")
                            nc.tensor.matmul(p[:kwd, :cw], ksb[h][:, k0 : k0 + kwd],
                                             qsb[h][:, c0 : c0 + cw], start=True, stop=True)
                            nc.vector.scalar_tensor_tensor(sx[:kwd, c0 : c0 + cw],
                                                           steps_t[kt_i][:kwd, c0 : c0 + cw],
                                                           -SLOPES[h], p[:kwd, :cw],
                                                           OP.mult, OP.add)
                        pt = w6.tile([128, TK], bf, tag="w6", name="w6")
                        nc.scalar.activation(pt[:kwd, :T], sx[:kwd, :], AF.Exp)
                        pts.append(pt)
                    rb = f6.tile([128, T], f32, tag="f6", name="f6")
                    for c0, cw in TCH:
                        pd = pk.tile([128, 512], f32, tag="pk", name="pk")
                        for kt_i, (k0, kwd) in enumerate(KT):
                            nc.tensor.matmul(pd[:, :cw], ones_b128[:kwd, :],
                                             pts[kt_i][:kwd, c0 : c0 + cw],
                                             start=(kt_i == 0), stop=(kt_i == 4))
                        nc.vector.reciprocal(rb[:, c0 : c0 + cw], pd[:, :cw])
                    at = w6.tile([128, TK], bf, tag="w6", name="w6")
                    for c0, cw in TCH:
                        p = pk.tile([128, 512], f32, tag="pk", name="pk")
                        for kt_i, (k0, kwd) in enumerate(KT):
                            nc.tensor.matmul(p[:, :cw], vsb[kt_i][:kwd, h * 128 : (h + 1) * 128],
                                             pts[kt_i][:kwd, c0 : c0 + cw],
                                             start=(kt_i == 0), stop=(kt_i == 4))
                        nc.vector.tensor_tensor(at[:, c0 : c0 + cw], p[:, :cw],
                                                rb[:, c0 : c0 + cw], OP.mult)
                    attn.append(at)
                def ca_load(mat):
                    ws = []
                    for i in range(DI):
                        w = wg.tile([128, D], bf, tag="wg", name="wg")
                        nc.sync.dma_start(w[:], ca_wqkvT_c[l, mat, i])
                        ws.append(w)
                    return ws

                def ca_proj(ws, src_tiles, src_off, chunks, bias_off, scale):
                    outs = []
                    for ot in range(DI):
                        dst = w6.tile([128, TK], bf, tag="w6", name="w6")
                        for c0, cw in chunks:
                            p = pk.tile([128, 512], f32, tag="pk", name="pk")
                            for i in range(DI):
                                nc.tensor.matmul(p[:, :cw], ws[i][:, ot * 128 : (ot + 1) * 128],
                                                 src_tiles[i][:, src_off + c0 : src_off + c0 + cw],
                                                 start=(i == 0), stop=(i == DI - 1))
                            nc.scalar.activation(dst[:, c0 : c0 + cw], p[:, :cw], AF.Identity,
                                                 bias=ca_bq[:, bias_off + ot : bias_off + ot + 1],
                                                 scale=scale)
                        outs.append(dst)
                    return outs

                ca_kw = ca_load(1)
                proj_res(b, sa_woT_c[l], sa_bo_t[:], attn)
                layernorm(b, lng[0][:], lnb[0][:])

                # ================= cross-attention =================
                # k/v first: they depend only on memory, so they overlap LN1
                ks_ca = ca_proj(ca_kw, mx1, 0, KCH, DI, 1.0)
                vs_ca = ca_proj(ca_load(2), mx1, 0, KCH, 2 * DI, 1.0)
                qs_ca = ca_proj(ca_load(0), hb[b], 1, TCH, 0, INV)

                ca_attn = []
                for h in range(H):
                    kh, vh, qh = ks_ca[h], vs_ca[h], qs_ca[h]
                    ka = f6.tile([128, 1], f32, tag="ka", name="ka")
                    nc.vector.tensor_copy(ka[:], kh[:, 0:1])
                    va = f6.tile([128, 1], f32, tag="ka", name="va")
                    nc.vector.tensor_copy(va[:], vh[:, 0:1])
                    kd = w6.tile([128, TK], bf, tag="w6", name="w6")
                    nc.vector.tensor_scalar_sub(kd[:, :T], kh[:, 1:], ka[:])
                    e = w6.tile([128, TK], bf, tag="w6", name="w6")
                    nc.vector.tensor_tensor(e[:, :T], qh[:, :T], kd[:, :T], OP.mult)
                    wm = w6.tile([128, TK], bf, tag="w6", name="w6")
                    for c0, cw in TCH:
                        pd = pk.tile([128, 512], f32, tag="pk", name="pk")
                        nc.tensor.matmul(pd[:, :cw], ones_b128[:], e[:, c0 : c0 + cw],
                                         start=True, stop=True)
                        nc.scalar.activation(wm[:, c0 : c0 + cw], pd[:, :cw], AF.Sigmoid)
                    vd = w6.tile([128, TK], bf, tag="w6", name="w6")
                    nc.vector.tensor_scalar_sub(vd[:, :T], vh[:, 1:], va[:])
                    at = w6.tile([128, TK], bf, tag="w6", name="w6")
                    nc.vector.tensor_tensor(at[:, :T], vd[:, :T], wm[:, :T], OP.mult)
                    nc.vector.tensor_scalar_add(at[:, :T], at[:, :T], va[:])
                    ca_attn.append(at)
                proj_res(b, ca_woT_c[l], ca_bo_t[:], ca_attn)
                hq = [hqp.tile([128, TK], bf, tag="hqp", name="hqp") for _ in range(DI)]
                w1pre = []
                for i in range(DI):
                    w = wg.tile([128, D], bf, tag="wg", name="wg")
                    nc.sync.dma_start(w[:], ff_w1T_c[l, 0, i])
                    w1pre.append(w)
                layernorm(b, lng[1][:], lnb[1][:], hq)

                # ================= FFN =================
                for half in range(2):
                    ffa = []
                    for g2 in range(2):
                        gi = half * 2 + g2
                        if gi == 0:
                            w1s = w1pre
                        else:
                            w1s = []
                            for i in range(DI):
                                w = wg.tile([128, D], bf, tag="wg", name="wg")
                                nc.sync.dma_start(w[:], ff_w1T_c[l, gi, i])
                                w1s.append(w)
                        for ot in range(DI):
                            o = gi * DI + ot
                            dst = fa.tile([128, TK], bf, tag="fa", name="fa")
                            for c0, cw in TCH:
                                p = pk.tile([128, 512], f32, tag="pk", name="pk")
                                for i in range(DI):
                                    nc.tensor.matmul(p[:, :cw], w1s[i][:, ot * 128 : (ot + 1) * 128],
                                                     hq[i][:, c0 : c0 + cw],
                                                     start=(i == 0), stop=(i == DI - 1))
                                nc.scalar.activation(dst[:, c0 : c0 + cw], p[:, :cw], AF.Relu,
                                                     bias=f_b1[:, o : o + 1])
                            ffa.append(dst)
                    w2s = []
                    for ii in range(16):
                        w = wg.tile([128, D], bf, tag="wg", name="wg")
                        nc.sync.dma_start(w[:], ff_w2T_c[l, half, ii])
                        w2s.append(w)
                    for o in range(DI):
                        for c0, cw in TCH:
                            p = pk.tile([128, 512], f32, tag="pk", name="pk")
                            for ii in range(16):
                                nc.tensor.matmul(p[:, :cw], w2s[ii][:, o * 128 : (o + 1) * 128],
                                                 ffa[ii][:, c0 : c0 + cw],
                                                 start=(ii == 0), stop=(ii == 15))
                            if half == 0:
                                nc.vector.scalar_tensor_tensor(
                                    hb[b][o][:, 1 + c0 : 1 + c0 + cw], p[:, :cw],
                                    f_b2[:, o : o + 1], hb[b][o][:, 1 + c0 : 1 + c0 + cw],
                                    OP.add, OP.add)
                            else:
                                nc.vector.tensor_tensor(hb[b][o][:, 1 + c0 : 1 + c0 + cw],
                                                        p[:, :cw],
                                                        hb[b][o][:, 1 + c0 : 1 + c0 + cw],
                                                        OP.add)
                layernorm(b, lng[2][:], lnb[2][:])

            # ---------- output projection ----------
            wo_t = []
            for i in range(DI):
                w = wg.tile([128, IN], bf, tag="wgout", name="wgout")
                nc.sync.dma_start(w[:], w_outT_c[i])
                wo_t.append(w)
            ot_ = sm.tile([IN, T], f32, tag=f"osb{b}", name=f"osb{b}")
            for c0, cw in TCH:
                p = pk.tile([128, 512], f32, tag="pk", name="pk")
                for i in range(DI):
                    nc.tensor.matmul(p[:IN, :cw], wo_t[i][:], hb[b][i][:, 1 + c0 : 1 + c0 + cw],
                                     start=(i == 0), stop=(i == DI - 1))
                nc.scalar.activation(ot_[:, c0 : c0 + cw], p[:IN, :cw], AF.Identity, bias=bo_t[:])
            nc.sync.dma_start(out_d[b], ot_[:])

        for _pool in (pk, sm, wg, sqp, f6, vs, hqp, fa, w6, res):
            _pool.release()

    nc.compile()
    return nc


def _prep_host(inputs):
    """Build the 8 per-core input maps from full inputs."""
    f32 = np.float32

    def b16(a):
        return np.ascontiguousarray(np.asarray(a, f32)).astype(bf16np)

    def tiled(vec, n):          # [n*128] -> [128, n] (col j = tile j)
        return np.ascontiguousarray(np.asarray(vec, f32).reshape(n, 128).T)

    x = np.asarray(inputs["x"], f32)
    memory = np.asarray(inputs["memory"], f32)
    ts = np.asarray(inputs["timesteps"])
    pe = np.asarray(inputs["pe"], f32)

    half = D // 2
    expo = np.exp(-math.log(10000.0) * np.arange(half, dtype=f32) / (half - 1.0))
    efm = np.concatenate([expo, expo]) / (2 * np.pi)
    phs = np.concatenate([np.zeros(half, f32), np.full(half, 0.25, f32)])

    # alibi steps, kt-tiled: steps_d[kt, k-k0, q]; bias[h] = -slope_h * steps
    di = np.arange(T)[:, None] - np.arange(T)[None, :]
    steps = np.where(di >= 0, di // PERIOD, (-di - 1) // PERIOD).astype(f32)  # [q, j]
    stepsT = np.zeros((TK, T), f32)
    stepsT[1:, :] = steps.T                     # [1+j, q]; row 0 (adapter) = 0
    steps_d = np.zeros((5, 128, T), f32)
    for kt_i, (k0, kwd) in enumerate(KT):
        steps_d[kt_i, :kwd] = stepsT[k0 : k0 + kwd]

    qkv_bias = {}
    for nm in ("sa", "ca"):
        bq = np.asarray(inputs[f"{nm}_bqkv"], f32).copy()      # [L, 3D]
        bq[:, :D] *= INV                                       # pre-scale q bias
        qkv_bias[nm] = np.stack([np.stack([tiled(bq[l, k * 128 : (k + 1) * 128], 1)[:, 0]
                                           for k in range(3 * DI)], axis=1)
                                 for l in range(L)])           # [L,128,24]

    def qkv_c(w):  # [L, 3D, D] -> [L, 3, DI, 128, D] tile-contiguous
        wT = np.asarray(w, f32).transpose(0, 2, 1)             # [L, D, 3D]
        return b16(wT.reshape(L, DI, 128, 3, D).transpose(0, 3, 1, 2, 4))

    common = {
        "tsf": None, "xT": None, "memT_c": None,
        "efm": tiled(efm, DI), "phs": tiled(phs, DI),
        "peT_c": np.ascontiguousarray(
            (pe.T + np.asarray(inputs["b_in"], f32)[:, None]).reshape(DI, 128, T)),
        "w_inT": b16(np.asarray(inputs["W_in"], f32).T),
        "te_w1T_c": b16(np.asarray(inputs["te_W1"], f32).T.reshape(DI, 128, D)),
        "te_w2T_c": b16(np.asarray(inputs["te_W2"], f32).T.reshape(DI, 128, D)),
        "te_b1t": tiled(inputs["te_b1"], DI),
        "te_b2t": tiled(inputs["te_b2"], DI),
        "sa_wqkvT_c": qkv_c(inputs["sa_Wqkv"]),
        "sa_bqkvt": qkv_bias["sa"],
        "sa_bvrow": b16(np.asarray(inputs["sa_bqkv"], f32)[:, 2 * D :][:, None, :]),
        "sa_woT_c": b16(np.asarray(inputs["sa_Wo"], f32).transpose(0, 2, 1).reshape(L, DI, 128, D)),
        "sa_bot": np.stack([tiled(np.asarray(inputs["sa_bo"], f32)[l], DI) for l in range(L)]),
        "ca_wqkvT_c": qkv_c(inputs["ca_Wqkv"]),
        "ca_bqkvt": qkv_bias["ca"],
        "ca_woT_c": b16(np.asarray(inputs["ca_Wo"], f32).transpose(0, 2, 1).reshape(L, DI, 128, D)),
        "ca_bot": np.stack([tiled(np.asarray(inputs["ca_bo"], f32)[l], DI) for l in range(L)]),
        "ff_w1T_c": b16(np.asarray(inputs["ff_W1"], f32).transpose(0, 2, 1)
                        .reshape(L, DI, 128, 4, D).transpose(0, 3, 1, 2, 4)),
        "ff_b1t": np.stack([tiled(np.asarray(inputs["ff_b1"], f32)[l], DFI) for l in range(L)]),
        "ff_w2T_c": b16(np.asarray(inputs["ff_W2"], f32).transpose(0, 2, 1)
                        .reshape(L, 2, 16, 128, D)),
        "ff_b2t": np.stack([tiled(np.asarray(inputs["ff_b2"], f32)[l], DI) for l in range(L)]),
        "lngt": np.stack([np.stack([tiled(np.asarray(inputs[f"ln{k+1}_g"], f32)[l], DI)
                                    for k in range(3)]) for l in range(L)]),
        "lnbt": np.stack([np.stack([tiled(np.asarray(inputs[f"ln{k+1}_b"], f32)[l], DI)
                                    for k in range(3)]) for l in range(L)]),
        "steps_d": steps_d.astype(bf16np),
        "w_outT_c": b16(np.asarray(inputs["W_out"], f32).T.reshape(DI, 128, IN)),
        "b_out": np.asarray(inputs["b_out"], f32)[:, None],
    }

    in_maps = []
    for c in range(NC):
        b0 = c * BC
        m = dict(common)
        m["xT"] = b16(x[b0 : b0 + BC].transpose(0, 2, 1))
        m["memT_c"] = b16(memory[b0 : b0 + BC].transpose(0, 2, 1).reshape(BC, DI, 128, T))
        m["tsf"] = np.asarray(ts[b0 : b0 + BC], f32)[None, :]
        in_maps.append(m)
    return in_maps


def kernel(**inputs):
    from concourse.bass_utils import run_bass_kernel_spmd

    if "nc" not in _cache:
        _cache["nc"] = _build()
    nc = _cache["nc"]
    in_maps = _prep_host(inputs)
    res = run_bass_kernel_spmd(nc, in_maps, core_ids=list(range(NC)))
    out = np.empty((B, T, IN), np.float32)
    for c in range(NC):
        out[c * BC : (c + 1) * BC] = res.results[c]["out"].transpose(0, 2, 1)
    return out


# revision 15
# speedup vs baseline: 1.0465x; 1.0465x over previous
import math
import sys

sys.path.insert(0, "/opt/trn_rl_repo")

import numpy as np
import ml_dtypes

bf16np = ml_dtypes.bfloat16

# ---------------- problem constants (hardcoded; kernel.py must be self-contained) ----
B, T, S, D, H, L, DFF, IN, PERIOD = 16, 600, 600, 1024, 8, 8, 4096, 52, 25
HD = D // H          # 128
NC = 8               # cores
BC = B // NC         # 2 batches per core
DI = D // 128        # 8 i-tiles
DFI = DFF // 128     # 32
INV = 1.0 / math.sqrt(HD)
TK = 1 + T           # 601 keys (adapter + T)
# chunks of the token dim (>=256 wide keeps LDWEIGHTS hidden; <=512 fits a psum bank)
TCH = [(0, 300), (300, 300)]
KCH = [(0, 301), (301, 300)]         # 601-wide
KT = [(0, 128), (128, 128), (256, 128), (384, 128), (512, 89)]  # key tiles of 601
SLOPES = [0.5 ** (h + 1) for h in range(H)]

_cache = {}


def _build():
    """Build the per-core Bass graph (SPMD; same program all 8 cores)."""
    from concourse import bacc, mybir
    import concourse.bass as bass
    import concourse.tile as tile

    f32 = mybir.dt.float32
    bf = mybir.dt.bfloat16
    i32 = mybir.dt.int32
    AF = mybir.ActivationFunctionType
    OP = mybir.AluOpType

    nc = bacc.Bacc("TRN2", target_bir_lowering=False, debug=False, num_devices=NC)

    def din(name, shape, dt=f32):
        return nc.dram_tensor(name, shape, dt, kind="ExternalInput").ap()

    # ---- DRAM inputs (host-prepped layouts; *_c = tile-contiguous) ----
    xT = din("xT", [BC, IN, T], bf)              # x transposed, bf16
    memT_c = din("memT_c", [BC, DI, 128, T], bf)
    tsf = din("tsf", [1, BC])                    # timesteps as f32
    efm = din("efm", [128, DI])                  # e/(2pi) tiled per i-tile col
    phs = din("phs", [128, DI])                  # phase (0 / .25)
    peT_c = din("peT_c", [DI, 128, T])           # pe.T + b_in  (f32)
    w_inT = din("w_inT", [IN, D], bf)
    te_w1T_c = din("te_w1T_c", [DI, 128, D], bf)
    te_w2T_c = din("te_w2T_c", [DI, 128, D], bf)
    te_b1t = din("te_b1t", [128, DI])
    te_b2t = din("te_b2t", [128, DI])
    sa_wqkvT_c = din("sa_wqkvT_c", [L, 3, DI, 128, D], bf)
    sa_bqkvt = din("sa_bqkvt", [L, 128, 3 * DI])  # pre-tiled [128, 24]; q-part prescaled by INV
    sa_bvrow = din("sa_bvrow", [L, 1, D], bf)     # v-bias as row (for ones-MM trick)
    sa_woT_c = din("sa_woT_c", [L, DI, 128, D], bf)
    sa_bot = din("sa_bot", [L, 128, DI])
    ca_wqkvT_c = din("ca_wqkvT_c", [L, 3, DI, 128, D], bf)
    ca_bqkvt = din("ca_bqkvt", [L, 128, 3 * DI])
    ca_woT_c = din("ca_woT_c", [L, DI, 128, D], bf)
    ca_bot = din("ca_bot", [L, 128, DI])
    ff_w1T_c = din("ff_w1T_c", [L, 4, DI, 128, D], bf)
    ff_b1t = din("ff_b1t", [L, 128, DFI])
    ff_w2T_c = din("ff_w2T_c", [L, 2, 16, 128, D], bf)
    ff_b2t = din("ff_b2t", [L, 128, DI])
    lngt = din("lngt", [L, 3, 128, DI])
    lnbt = din("lnbt", [L, 3, 128, DI])
    steps_d = din("steps_d", [5, 128, T], bf)    # alibi steps, kt-tiled [k, q]
    w_outT_c = din("w_outT_c", [DI, 128, IN], bf)
    b_out = din("b_out", [IN, 1])
    out_d = nc.dram_tensor("out", [BC, IN, T], f32, kind="ExternalOutput").ap()

    with tile.TileContext(nc) as tc:
        res = tc.alloc_tile_pool(name="res", bufs=1)      # persistent
        w6 = tc.alloc_tile_pool(name="w6", bufs=28)       # bf16 [128,601] q/k/pt/attn ws
        fa = tc.alloc_tile_pool(name="fa", bufs=17)       # bf16 [128,601] ffa tiles
        hqp = tc.alloc_tile_pool(name="hqp", bufs=10)     # bf16 [128,601] LN targets
        vs = tc.alloc_tile_pool(name="vs", bufs=6)        # bf16 [128,1024] V tiles
        f6 = tc.alloc_tile_pool(name="f6", bufs=8)        # f32 [128,600] workspace
        sqp = tc.alloc_tile_pool(name="sqp", bufs=8)      # bf16 [128,600] LN squares
        wg = tc.alloc_tile_pool(name="wg", bufs=17)       # bf16 [128,1024] weights
        sm = tc.alloc_tile_pool(name="sm", bufs=1)        # small persistents
        pk = tc.alloc_tile_pool(name="pk", bufs=8, space="PSUM")

        # persistent bf16 residual stream (adapter col 0) - ONE batch at a time
        hb1 = [res.tile([128, TK], bf, tag=f"hb_{i}", name=f"hb_{i}") for i in range(DI)]
        hb = [hb1 for _ in range(BC)]
        mx1 = [res.tile([128, TK], bf, tag=f"mx_{i}", name=f"mx_{i}") for i in range(DI)]

        ones_b = sm.tile([1, 128], bf, tag="ones_b", name="ones_b")
        nc.vector.memset(ones_b[:], 1.0)
        ones_b128 = sm.tile([128, 128], bf, tag="ones_b128", name="ones_b128")
        nc.vector.memset(ones_b128[:], 1.0)
        ones_f = sm.tile([1, 128], f32, tag="ones_f", name="ones_f")
        nc.vector.memset(ones_f[:], 1.0)
        eft = sm.tile([128, DI], f32, tag="eft", name="eft")
        nc.sync.dma_start(eft[:], efm[:])
        pht = sm.tile([128, DI], f32, tag="pht", name="pht")
        nc.sync.dma_start(pht[:], phs[:])
        tst = sm.tile([1, BC], f32, tag="tst", name="tst")
        eps_t = sm.tile([128, 1], f32, tag="eps_t", name="eps_t")
        nc.vector.memset(eps_t[:], 1e-5)
        nc.sync.dma_start(tst[:], tsf[:])
        adp = [sm.tile([128, BC], bf, tag=f"adp{i}", name=f"adp{i}") for i in range(DI)]  # adapter bf16
        steps_t = [sm.tile([128, T], bf, tag=f"steps{k}", name=f"steps{k}") for k in range(5)]
        for k in range(5):
            nc.sync.dma_start(steps_t[k][:], steps_d[k])

        # ---------- timestep embedding ----------
        ptb = pk.tile([128, BC], f32, tag="pk", name="ptb")
        nc.tensor.matmul(ptb[:], ones_f[:], tst[:], start=True, stop=True)  # t bcast f32
        temb = []
        for i in range(DI):
            y = sm.tile([128, BC], f32, tag=f"y{i}", name=f"y{i}")
            nc.vector.tensor_scalar_mul(y[:], ptb[:], eft[:, i : i + 1])
            nc.vector.tensor_scalar_add(y[:], y[:], pht[:, i : i + 1])
            yi = sm.tile([128, BC], i32, tag=f"yi{i}", name=f"yi{i}")
            nc.vector.tensor_copy(yi[:], y[:])
            yr = sm.tile([128, BC], f32, tag=f"yr{i}", name=f"yr{i}")
            nc.vector.tensor_copy(yr[:], yi[:])
            fr = sm.tile([128, BC], f32, tag=f"fr{i}", name=f"fr{i}")
            nc.vector.tensor_sub(fr[:], y[:], yr[:])
            tb = sm.tile([128, BC], bf, tag=f"tb{i}", name=f"tb{i}")
            nc.scalar.activation(tb[:], fr[:], AF.Sin, scale=2 * math.pi)
            temb.append(tb)

        def mlp1024(wT_c, bt_d, ins, act, outs_tag):
            """[D,D] proj on BC-wide f-major input tiles. Returns 8 bf16 [128,BC] tiles."""
            bt = sm.tile([128, DI], f32, tag=outs_tag + "_b", name=outs_tag + "_b")
            nc.sync.dma_start(bt[:], bt_d[:])
            ws = []
            for i in range(DI):
                w = wg.tile([128, D], bf, tag="wg", name="wg")
                nc.sync.dma_start(w[:], wT_c[i])
                ws.append(w)
            outs = []
            for o in range(DI):
                p = pk.tile([128, BC], f32, tag="pk", name="pmlp")
                for i in range(DI):
                    nc.tensor.matmul(p[:], ws[i][:, o * 128 : (o + 1) * 128], ins[i][:],
                                     start=(i == 0), stop=(i == DI - 1))
                ob = sm.tile([128, BC], bf, tag=f"{outs_tag}{o}", name=f"{outs_tag}{o}")
                nc.scalar.activation(ob[:], p[:], act, bias=bt[:, o : o + 1])
                outs.append(ob)
            return outs

        z1 = mlp1024(te_w1T_c, te_b1t, temb, AF.Silu, "z1")
        z2 = mlp1024(te_w2T_c, te_b2t, z1, AF.Identity, "z2")
        for i in range(DI):
            nc.vector.tensor_copy(adp[i][:], z2[i][:])

        # ---------- helpers ----------
        def layernorm(b, g_ap, b_ap, tgt=None):
            """LN over features of X held in hb[b][:, 1:] (bf16). Sums via ones-matmul
            reduction. If tgt given, apply writes tgt and hb is back-filled off the
            critical path; else apply is in place on hb."""
            sq = []
            for o in range(DI):
                s = sqp.tile([128, T], bf, tag="sqp", name="sqp")
                nc.vector.tensor_mul(s[:], hb[b][o][:, 1:], hb[b][o][:, 1:])
                sq.append(s)
            m = f6.tile([128, T], f32, tag="f6", name="f6")
            rstd = f6.tile([128, T], f32, tag="f6", name="f6")
            mr = f6.tile([128, T], f32, tag="f6", name="f6")
            for c0, cw in TCH:
                pS = pk.tile([128, 512], f32, tag="pk", name="pk")
                for o in range(DI):
                    nc.tensor.matmul(pS[:, :cw], ones_b128[:],
                                     hb[b][o][:, 1 + c0 : 1 + c0 + cw],
                                     start=(o == 0), stop=(o == DI - 1))
                pS2 = pk.tile([128, 512], f32, tag="pk", name="pk")
                for o in range(DI):
                    nc.tensor.matmul(pS2[:, :cw], ones_b128[:], sq[o][:, c0 : c0 + cw],
                                     start=(o == 0), stop=(o == DI - 1))
                nc.vector.tensor_scalar_mul(m[:, c0 : c0 + cw], pS[:, :cw], 1.0 / D)
                m2 = f6.tile([128, T], f32, tag="f6", name="f6")
                nc.vector.tensor_tensor(m2[:, :cw], m[:, c0 : c0 + cw], m[:, c0 : c0 + cw], OP.mult)
                var = f6.tile([128, T], f32, tag="f6", name="f6")
                nc.vector.scalar_tensor_tensor(var[:, :cw], pS2[:, :cw], 1.0 / D, m2[:, :cw],
                                               OP.mult, OP.subtract)
                sd = f6.tile([128, T], f32, tag="f6", name="f6")
                nc.scalar.activation(sd[:, :cw], var[:, :cw], AF.Sqrt, bias=eps_t[:])
                nc.vector.reciprocal(rstd[:, c0 : c0 + cw], sd[:, :cw])
                nc.vector.tensor_tensor(mr[:, c0 : c0 + cw], m[:, c0 : c0 + cw],
                                        rstd[:, c0 : c0 + cw], OP.mult)
            for o in range(DI):
                dst = tgt[o] if tgt is not None else None
                for c0, cw in TCH:
                    t1 = f6.tile([128, T], f32, tag="f6", name="f6")
                    nc.vector.tensor_tensor(t1[:, :cw], hb[b][o][:, 1 + c0 : 1 + c0 + cw],
                                            rstd[:, c0 : c0 + cw], OP.mult)
                    nc.vector.tensor_tensor(t1[:, :cw], t1[:, :cw], mr[:, c0 : c0 + cw],
                                            OP.subtract)
                    if dst is not None:
                        nc.scalar.activation(dst[:, c0 : c0 + cw], t1[:, :cw], AF.Identity,
                                             bias=b_ap[:, o : o + 1], scale=g_ap[:, o : o + 1])
                    else:
                        nc.scalar.activation(hb[b][o][:, 1 + c0 : 1 + c0 + cw], t1[:, :cw],
                                             AF.Identity,
                                             bias=b_ap[:, o : o + 1], scale=g_ap[:, o : o + 1])
                if dst is not None:
                    nc.vector.tensor_copy(hb[b][o][:, 1:], dst[:, :T])

        def proj_res(b, wT_c_l, bot_ap, rhs_tiles):
            """out-proj [D,D] + bias + residual into hf[b] (X pre-LN)."""
            ws = []
            for i in range(DI):
                w = wg.tile([128, D], bf, tag="wg", name="wg")
                nc.sync.dma_start(w[:], wT_c_l[i])
                ws.append(w)
            for o in range(DI):
                for c0, cw in TCH:
                    p = pk.tile([128, 512], f32, tag="pk", name="pk")
                    for i in range(DI):
                        nc.tensor.matmul(p[:, :cw], ws[i][:, o * 128 : (o + 1) * 128],
                                         rhs_tiles[i][:, c0 : c0 + cw],
                                         start=(i == 0), stop=(i == DI - 1))
                    nc.vector.scalar_tensor_tensor(hb[b][o][:, 1 + c0 : 1 + c0 + cw], p[:, :cw],
                                                   bot_ap[:, o : o + 1],
                                                   hb[b][o][:, 1 + c0 : 1 + c0 + cw],
                                                   OP.add, OP.add)

        # per-layer bias tiles (re-DMAed each (b, l))
        sa_bq = sm.tile([128, 3 * DI], f32, tag="sa_bq", name="sa_bq")
        ca_bq = sm.tile([128, 3 * DI], f32, tag="ca_bq", name="ca_bq")
        sa_bo_t = sm.tile([128, DI], f32, tag="sa_bo_t", name="sa_bo_t")
        ca_bo_t = sm.tile([128, DI], f32, tag="ca_bo_t", name="ca_bo_t")
        f_b1 = sm.tile([128, DFI], f32, tag="f_b1", name="f_b1")
        f_b2 = sm.tile([128, DI], f32, tag="f_b2", name="f_b2")
        lng = [sm.tile([128, DI], f32, tag=f"lng{k}", name=f"lng{k}") for k in range(3)]
        lnb = [sm.tile([128, DI], f32, tag=f"lnb{k}", name=f"lnb{k}") for k in range(3)]
        bvr = sm.tile([1, D], bf, tag="bvr", name="bvr")
        bo_t = sm.tile([IN, 1], f32, tag="bo_t", name="bo_t")
        nc.sync.dma_start(bo_t[:], b_out[:])

        # ================= batch-serial main =================
        for b in range(BC):
            # ---------- input projection + pe ----------
            xb = sm.tile([IN, T], bf, tag="xb", name="xb")
            nc.sync.dma_start(xb[:], xT[b])
            w_in_t = sm.tile([IN, D], bf, tag="w_in_t", name="w_in_t")
            nc.sync.dma_start(w_in_t[:], w_inT[:])
            for o in range(DI):
                pe_t = f6.tile([128, T], f32, tag="f6", name="f6")
                nc.sync.dma_start(pe_t[:], peT_c[o])
                for c0, cw in TCH:
                    p = pk.tile([128, 512], f32, tag="pk", name="pk")
                    nc.tensor.matmul(p[:, :cw], w_in_t[:, o * 128 : (o + 1) * 128],
                                     xb[:, c0 : c0 + cw], start=True, stop=True)
                    nc.vector.tensor_tensor(hb[b][o][:, 1 + c0 : 1 + c0 + cw], p[:, :cw],
                                            pe_t[:, c0 : c0 + cw], OP.add)
                nc.vector.tensor_copy(hb[b][o][:, 0:1], adp[o][:, b : b + 1])
                nc.sync.dma_start(mx1[o][:, 1:], memT_c[b, o])
                nc.vector.tensor_copy(mx1[o][:, 0:1], adp[o][:, b : b + 1])

            for l in range(L):
                nc.sync.dma_start(sa_bq[:], sa_bqkvt[l])
                nc.sync.dma_start(ca_bq[:], ca_bqkvt[l])
                nc.sync.dma_start(sa_bo_t[:], sa_bot[l])
                nc.sync.dma_start(ca_bo_t[:], ca_bot[l])
                nc.sync.dma_start(f_b1[:], ff_b1t[l])
                nc.sync.dma_start(f_b2[:], ff_b2t[l])
                for k in range(3):
                    nc.sync.dma_start(lng[k][:], lngt[l, k])
                    nc.sync.dma_start(lnb[k][:], lnbt[l, k])
                nc.sync.dma_start(bvr[:], sa_bvrow[l])

                # ================= self-attention =================
                def sa_proj(mat, src_off, chunks, bias_off, scale):
                    ws = []
                    for i in range(DI):
                        w = wg.tile([128, D], bf, tag="wg", name="wg")
                        nc.sync.dma_start(w[:], sa_wqkvT_c[l, mat, i])
                        ws.append(w)
                    outs = []
                    for ot in range(DI):
                        dst = w6.tile([128, TK], bf, tag="w6", name="w6")
                        for c0, cw in chunks:
                            p = pk.tile([128, 512], f32, tag="pk", name="pk")
                            for i in range(DI):
                                nc.tensor.matmul(p[:, :cw], ws[i][:, ot * 128 : (ot + 1) * 128],
                                                 hb[b][i][:, src_off + c0 : src_off + c0 + cw],
                                                 start=(i == 0), stop=(i == DI - 1))
                            nc.scalar.activation(dst[:, c0 : c0 + cw], p[:, :cw], AF.Identity,
                                                 bias=sa_bq[:, bias_off + ot : bias_off + ot + 1],
                                                 scale=scale)
                        outs.append(dst)
                    return outs

                qsb = sa_proj(0, 1, TCH, 0, INV)
                ksb = sa_proj(1, 0, KCH, DI, 1.0)
                # V transposed ([key, head*hd]) via hx-stationary matmuls
                vw = []
                for i in range(DI):
                    w = wg.tile([128, D], bf, tag="wg", name="wg")
                    nc.sync.dma_start(w[:], sa_wqkvT_c[l, 2, i])
                    vw.append(w)
                vsb = [vs.tile([128, 1024], bf, tag="vs", name="vs") for _ in range(5)]
                for vc in range(2):
                    vc0 = vc * 512
                    for kt_i, (k0, kwd) in enumerate(KT):
                        p = pk.tile([128, 512], f32, tag="pk", name="pk")
                        for i in range(DI):
                            nc.tensor.matmul(p[:kwd, :], hb[b][i][:, k0 : k0 + kwd],
                                             vw[i][:, vc0 : vc0 + 512],
                                             start=(i == 0), stop=False)
                        nc.tensor.matmul(p[:kwd, :], ones_b[:, :kwd],
                                         bvr[:, vc0 : vc0 + 512], start=False, stop=True)
                        nc.scalar.copy(vsb[kt_i][:kwd, vc0 : vc0 + 512], p[:kwd, :])

                attn = []
                for h in range(H):
                    pts = []
                    for kt_i, (k0, kwd) in enumerate(KT):
                        sx = f6.tile([128, T], f32, tag="f6", name="f6")
                        for c0, cw in TCH:
                            p = pk.tile([128, 512], f32, tag="pk", name="pk")
                            nc.tensor.matmul(p[:kwd, :cw], ksb[h][:, k0 : k0 + kwd],
                                             qsb[h][:, c0 : c0 + cw], start=True, stop=True)
                            nc.vector.scalar_tensor_tensor(sx[:kwd, c0 : c0 + cw],
                                                           steps_t[kt_i][:kwd, c0 : c0 + cw],
                                                           -SLOPES[h], p[:kwd, :cw],
                                                           OP.mult, OP.add)
                        pt = w6.tile([128, TK], bf, tag="w6", name="w6")
                        nc.scalar.activation(pt[:kwd, :T], sx[:kwd, :], AF.Exp)
                        pts.append(pt)
                    rb = f6.tile([128, T], f32, tag="f6", name="f6")
                    for c0, cw in TCH:
                        pd = pk.tile([128, 512], f32, tag="pk", name="pk")
                        for kt_i, (k0, kwd) in enumerate(KT):
                            nc.tensor.matmul(pd[:, :cw], ones_b128[:kwd, :],
                                             pts[kt_i][:kwd, c0 : c0 + cw],
                                             start=(kt_i == 0), stop=(kt_i == 4))
                        nc.vector.reciprocal(rb[:, c0 : c0 + cw], pd[:, :cw])
                    at = w6.tile([128, TK], bf, tag="w6", name="w6")
                    for c0, cw in TCH:
                        p = pk.tile([128, 512], f32, tag="pk", name="pk")
                        for kt_i, (k0, kwd) in enumerate(KT):
                            nc.tensor.matmul(p[:, :cw], vsb[kt_i][:kwd, h * 128 : (h + 1) * 128],
                                             pts[kt_i][:kwd, c0 : c0 + cw],
                                             start=(kt_i == 0), stop=(kt_i == 4))
                        nc.vector.tensor_tensor(at[:, c0 : c0 + cw], p[:, :cw],
                                                rb[:, c0 : c0 + cw], OP.mult)
                    attn.append(at)
                def ca_load(mat):
                    ws = []
                    for i in range(DI):
                        w = wg.tile([128, D], bf, tag="wg", name="wg")
                        nc.sync.dma_start(w[:], ca_wqkvT_c[l, mat, i])
                        ws.append(w)
                    return ws

                def ca_proj(ws, src_tiles, src_off, chunks, bias_off, scale):
                    outs = []
                    for ot in range(DI):
                        dst = w6.tile([128, TK], bf, tag="w6", name="w6")
                        for c0, cw in chunks:
                            p = pk.tile([128, 512], f32, tag="pk", name="pk")
                            for i in range(DI):
                                nc.tensor.matmul(p[:, :cw], ws[i][:, ot * 128 : (ot + 1) * 128],
                                                 src_tiles[i][:, src_off + c0 : src_off + c0 + cw],
                                                 start=(i == 0), stop=(i == DI - 1))
                            nc.scalar.activation(dst[:, c0 : c0 + cw], p[:, :cw], AF.Identity,
                                                 bias=ca_bq[:, bias_off + ot : bias_off + ot + 1],
                                                 scale=scale)
                        outs.append(dst)
                    return outs

                proj_res(b, sa_woT_c[l], sa_bo_t[:], attn)
                layernorm(b, lng[0][:], lnb[0][:])

                # ================= cross-attention =================
                # k/v first: they depend only on memory, so they overlap LN1
                ks_ca = ca_proj(ca_load(1), mx1, 0, KCH, DI, 1.0)
                vs_ca = ca_proj(ca_load(2), mx1, 0, KCH, 2 * DI, 1.0)
                qs_ca = ca_proj(ca_load(0), hb[b], 1, TCH, 0, INV)

                ca_attn = []
                for h in range(H):
                    kh, vh, qh = ks_ca[h], vs_ca[h], qs_ca[h]
                    ka = f6.tile([128, 1], f32, tag="ka", name="ka")
                    nc.vector.tensor_copy(ka[:], kh[:, 0:1])
                    va = f6.tile([128, 1], f32, tag="ka", name="va")
                    nc.vector.tensor_copy(va[:], vh[:, 0:1])
                    kd = w6.tile([128, TK], bf, tag="w6", name="w6")
                    nc.vector.tensor_scalar_sub(kd[:, :T], kh[:, 1:], ka[:])
                    e = w6.tile([128, TK], bf, tag="w6", name="w6")
                    nc.vector.tensor_tensor(e[:, :T], qh[:, :T], kd[:, :T], OP.mult)
                    wm = w6.tile([128, TK], bf, tag="w6", name="w6")
                    for c0, cw in TCH:
                        pd = pk.tile([128, 512], f32, tag="pk", name="pk")
                        nc.tensor.matmul(pd[:, :cw], ones_b128[:], e[:, c0 : c0 + cw],
                                         start=True, stop=True)
                        nc.scalar.activation(wm[:, c0 : c0 + cw], pd[:, :cw], AF.Sigmoid)
                    vd = w6.tile([128, TK], bf, tag="w6", name="w6")
                    nc.vector.tensor_scalar_sub(vd[:, :T], vh[:, 1:], va[:])
                    at = w6.tile([128, TK], bf, tag="w6", name="w6")
                    nc.vector.tensor_tensor(at[:, :T], vd[:, :T], wm[:, :T], OP.mult)
                    nc.vector.tensor_scalar_add(at[:, :T], at[:, :T], va[:])
                    ca_attn.append(at)
                proj_res(b, ca_woT_c[l], ca_bo_t[:], ca_attn)
                hq = [hqp.tile([128, TK], bf, tag="hqp", name="hqp") for _ in range(DI)]
                layernorm(b, lng[1][:], lnb[1][:], hq)

                # ================= FFN =================
                for half in range(2):
                    ffa = []
                    for g2 in range(2):
                        gi = half * 2 + g2
                        w1s = []
                        for i in range(DI):
                            w = wg.tile([128, D], bf, tag="wg", name="wg")
                            nc.sync.dma_start(w[:], ff_w1T_c[l, gi, i])
                            w1s.append(w)
                        for ot in range(DI):
                            o = gi * DI + ot
                            dst = fa.tile([128, TK], bf, tag="fa", name="fa")
                            for c0, cw in TCH:
                                p = pk.tile([128, 512], f32, tag="pk", name="pk")
                                for i in range(DI):
                                    nc.tensor.matmul(p[:, :cw], w1s[i][:, ot * 128 : (ot + 1) * 128],
                                                     hq[i][:, c0 : c0 + cw],
                                                     start=(i == 0), stop=(i == DI - 1))
                                nc.scalar.activation(dst[:, c0 : c0 + cw], p[:, :cw], AF.Relu,
                                                     bias=f_b1[:, o : o + 1])
                            ffa.append(dst)
                    w2s = []
                    for ii in range(16):
                        w = wg.tile([128, D], bf, tag="wg", name="wg")
                        nc.sync.dma_start(w[:], ff_w2T_c[l, half, ii])
                        w2s.append(w)
                    for o in range(DI):
                        for c0, cw in TCH:
                            p = pk.tile([128, 512], f32, tag="pk", name="pk")
                            for ii in range(16):
                                nc.tensor.matmul(p[:, :cw], w2s[ii][:, o * 128 : (o + 1) * 128],
                                                 ffa[ii][:, c0 : c0 + cw],
                                                 start=(ii == 0), stop=(ii == 15))
                            if half == 0:
                                nc.vector.scalar_tensor_tensor(
                                    hb[b][o][:, 1 + c0 : 1 + c0 + cw], p[:, :cw],
                                    f_b2[:, o : o + 1], hb[b][o][:, 1 + c0 : 1 + c0 + cw],
                                    OP.add, OP.add)
                            else:
                                nc.vector.tensor_tensor(hb[b][o][:, 1 + c0 : 1 + c0 + cw],
                                                        p[:, :cw],
                                                        hb[b][o][:, 1 + c0 : 1 + c0 + cw],
                                                        OP.add)
                layernorm(b, lng[2][:], lnb[2][:])

            # ---------- output projection ----------
            wo_t = []
            for i in range(DI):
                w = wg.tile([128, IN], bf, tag="wgout", name="wgout")
                nc.sync.dma_start(w[:], w_outT_c[i])
                wo_t.append(w)
            ot_ = sm.tile([IN, T], f32, tag=f"osb{b}", name=f"osb{b}")
            for c0, cw in TCH:
                p = pk.tile([128, 512], f32, tag="pk", name="pk")
                for i in range(DI):
                    nc.tensor.matmul(p[:IN, :cw], wo_t[i][:], hb[b][i][:, 1 + c0 : 1 + c0 + cw],
                                     start=(i == 0), stop=(i == DI - 1))
                nc.scalar.activation(ot_[:, c0 : c0 + cw], p[:IN, :cw], AF.Identity, bias=bo_t[:])
            nc.sync.dma_start(out_d[b], ot_[:])

        for _pool in (pk, sm, wg, sqp, f6, vs, hqp, fa, w6, res):
            _pool.release()

    nc.compile()
    return nc


def _prep_host(inputs):
    """Build the 8 per-core input maps from full inputs."""
    f32 = np.float32

    def b16(a):
        return np.ascontiguousarray(np.asarray(a, f32)).astype(bf16np)

    def tiled(vec, n):          # [n*128] -> [128, n] (col j = tile j)
        return np.ascontiguousarray(np.asarray(vec, f32).reshape(n, 128).T)

    x = np.asarray(inputs["x"], f32)
    memory = np.asarray(inputs["memory"], f32)
    ts = np.asarray(inputs["timesteps"])
    pe = np.asarray(inputs["pe"], f32)

    half = D // 2
    expo = np.exp(-math.log(10000.0) * np.arange(half, dtype=f32) / (half - 1.0))
    efm = np.concatenate([expo, expo]) / (2 * np.pi)
    phs = np.concatenate([np.zeros(half, f32), np.full(half, 0.25, f32)])

    # alibi steps, kt-tiled: steps_d[kt, k-k0, q]; bias[h] = -slope_h * steps
    di = np.arange(T)[:, None] - np.arange(T)[None, :]
    steps = np.where(di >= 0, di // PERIOD, (-di - 1) // PERIOD).astype(f32)  # [q, j]
    stepsT = np.zeros((TK, T), f32)
    stepsT[1:, :] = steps.T                     # [1+j, q]; row 0 (adapter) = 0
    steps_d = np.zeros((5, 128, T), f32)
    for kt_i, (k0, kwd) in enumerate(KT):
        steps_d[kt_i, :kwd] = stepsT[k0 : k0 + kwd]

    qkv_bias = {}
    for nm in ("sa", "ca"):
        bq = np.asarray(inputs[f"{nm}_bqkv"], f32).copy()      # [L, 3D]
        bq[:, :D] *= INV                                       # pre-scale q bias
        qkv_bias[nm] = np.stack([np.stack([tiled(bq[l, k * 128 : (k + 1) * 128], 1)[:, 0]
                                           for k in range(3 * DI)], axis=1)
                                 for l in range(L)])           # [L,128,24]

    def qkv_c(w):  # [L, 3D, D] -> [L, 3, DI, 128, D] tile-contiguous
        wT = np.asarray(w, f32).transpose(0, 2, 1)             # [L, D, 3D]
        return b16(wT.reshape(L, DI, 128, 3, D).transpose(0, 3, 1, 2, 4))

    common = {
        "tsf": None, "xT": None, "memT_c": None,
        "efm": tiled(efm, DI), "phs": tiled(phs, DI),
        "peT_c": np.ascontiguousarray(
            (pe.T + np.asarray(inputs["b_in"], f32)[:, None]).reshape(DI, 128, T)),
        "w_inT": b16(np.asarray(inputs["W_in"], f32).T),
        "te_w1T_c": b16(np.asarray(inputs["te_W1"], f32).T.reshape(DI, 128, D)),
        "te_w2T_c": b16(np.asarray(inputs["te_W2"], f32).T.reshape(DI, 128, D)),
        "te_b1t": tiled(inputs["te_b1"], DI),
        "te_b2t": tiled(inputs["te_b2"], DI),
        "sa_wqkvT_c": qkv_c(inputs["sa_Wqkv"]),
        "sa_bqkvt": qkv_bias["sa"],
        "sa_bvrow": b16(np.asarray(inputs["sa_bqkv"], f32)[:, 2 * D :][:, None, :]),
        "sa_woT_c": b16(np.asarray(inputs["sa_Wo"], f32).transpose(0, 2, 1).reshape(L, DI, 128, D)),
        "sa_bot": np.stack([tiled(np.asarray(inputs["sa_bo"], f32)[l], DI) for l in range(L)]),
        "ca_wqkvT_c": qkv_c(inputs["ca_Wqkv"]),
        "ca_bqkvt": qkv_bias["ca"],
        "ca_woT_c": b16(np.asarray(inputs["ca_Wo"], f32).transpose(0, 2, 1).reshape(L, DI, 128, D)),
        "ca_bot": np.stack([tiled(np.asarray(inputs["ca_bo"], f32)[l], DI) for l in range(L)]),
        "ff_w1T_c": b16(np.asarray(inputs["ff_W1"], f32).transpose(0, 2, 1)
                        .reshape(L, DI, 128, 4, D).transpose(0, 3, 1, 2, 4)),
        "ff_b1t": np.stack([tiled(np.asarray(inputs["ff_b1"], f32)[l], DFI) for l in range(L)]),
        "ff_w2T_c": b16(np.asarray(inputs["ff_W2"], f32).transpose(0, 2, 1)
                        .reshape(L, 2, 16, 128, D)),
        "ff_b2t": np.stack([tiled(np.asarray(inputs["ff_b2"], f32)[l], DI) for l in range(L)]),
        "lngt": np.stack([np.stack([tiled(np.asarray(inputs[f"ln{k+1}_g"], f32)[l], DI)
                                    for k in range(3)]) for l in range(L)]),
        "lnbt": np.stack([np.stack([tiled(np.asarray(inputs[f"ln{k+1}_b"], f32)[l], DI)
                                    for k in range(3)]) for l in range(L)]),
        "steps_d": steps_d.astype(bf16np),
        "w_outT_c": b16(np.asarray(inputs["W_out"], f32).T.reshape(DI, 128, IN)),
        "b_out": np.asarray(inputs["b_out"], f32)[:, None],
    }

    in_maps = []
    for c in range(NC):
        b0 = c * BC
        m = dict(common)
        m["xT"] = b16(x[b0 : b0 + BC].transpose(0, 2, 1))
        m["memT_c"] = b16(memory[b0 : b0 + BC].transpose(0, 2, 1).reshape(BC, DI, 128, T))
        m["tsf"] = np.asarray(ts[b0 : b0 + BC], f32)[None, :]
        in_maps.append(m)
    return in_maps


def kernel(**inputs):
    from concourse.bass_utils import run_bass_kernel_spmd

    if "nc" not in _cache:
        _cache["nc"] = _build()
    nc = _cache["nc"]
    in_maps = _prep_host(inputs)
    res = run_bass_kernel_spmd(nc, in_maps, core_ids=list(range(NC)))
    out = np.empty((B, T, IN), np.float32)
    for c in range(NC):
        out[c * BC : (c + 1) * BC] = res.results[c]["out"].transpose(0, 2, 1)
    return out
